# revision 24
# baseline (speedup 1.0000x reference)
"""Trainium2 Bass kernel for nn_DPFABase (DPFA knowledge-tracing attention).

Full-input contract: kernel(**inputs) takes the unsharded inputs and returns
the full [B, S] float32 output. Internally: data-parallel over batch across
8 NeuronCores (16 examples per core); the [V, H] embedding / beta / response
tables are replicated.

Per-core pipeline (v2 — SBUF-resident table, transposed gathers):
  1. One DMA loads the combined table (emb bf16 | beta r0 r1 pad) into SBUF
     in [v%128 partitions, v//128 stripes x 512B rows] layout; one DMA loads
     emb transposed [H, VPAD] bf16. Norms via DVE square + 79 ones-matmuls
     on PE -> PSUM [128, 79]; ACT rsqrt; 79 per-stripe tensor_scalar_mul
     normalize the table rows in place.
  2. Two SBUF-source dma_gathers (transpose=True, 8192 idxs x 512B rows)
     deliver, per batch of 8 examples, hist/next embeddings already
     transposed to [H, seq] plus aux rows (beta/r0/r1/pad) on partitions 0-3.
  3. Per example: QK matmuls (bf16) -> scoresT [s, q] PSUM; ACT exp with
     per-partition bias (-k*s + centering; per-q decay cancels in softmax);
     causal mask on diagonal tiles; PE row->col transposes of aux; num/den
     matmuls vs [mastery*pad | pad] -> [q, 2] PSUM.
  4. Final: ability = num/den, sigmoid(ability - beta_next), PE transpose,
     one DMA out.
"""
import numpy as np

B, S, H, V = 128, 512, 128, 10000
NCORES = 8
EXC = B // NCORES          # examples per core = 16
NBATCH = 8                 # gather batches per core
EXB = EXC // NBATCH        # examples per batch = 8
VPAD = 10112               # 79 * 128
NSTRIPE = VPAD // 128      # 79
ROW = 256                  # bf16 elements per augtab row (512 B)
ROWP = 132                 # packed table row: emb(128) + beta,r0,r1,pad
NIDX = EXB * 2 * S         # idxs per gather batch = 8192

_CACHE = {}


def _build_nc():
    import concourse.bacc as bacc
    import concourse.mybir as mybir
    from concourse.tile import TileContext

    f32 = mybir.dt.float32
    bf16 = mybir.dt.bfloat16
    i16 = mybir.dt.int16
    AF = mybir.ActivationFunctionType
    ALU = mybir.AluOpType

    nc = bacc.Bacc()

    table_in = nc.declare_dram_parameter("table_in", [128, NSTRIPE * ROWP], bf16, isOutput=False)
    embT_in = nc.declare_dram_parameter("embT_in", [128, VPAD], bf16, isOutput=False)
    idx_in = nc.declare_dram_parameter("idx_in", [NBATCH, 128, NIDX // 16], i16, isOutput=False)
    corr2_in = nc.declare_dram_parameter("corr2_in", [128, EXC * 4], bf16, isOutput=False)
    biaspp = nc.declare_dram_parameter("biaspp", [128, 4], f32, isOutput=False)
    causal = nc.declare_dram_parameter("causal", [128, 128], bf16, isOutput=False)
    identb = nc.declare_dram_parameter("identb", [128, 128], bf16, isOutput=False)
    identf = nc.declare_dram_parameter("identf", [128, 128], f32, isOutput=False)
    out = nc.declare_dram_parameter("out", [EXC, S], f32, isOutput=True)

    augtab = nc.dram_tensor("augtab", [VPAD, ROW], bf16)

    with TileContext(nc) as tc:
        with (
            tc.tile_pool(name="persist", bufs=1) as persist,
            tc.tile_pool(name="main", bufs=2) as main,
            tc.tile_pool(name="ej", bufs=2) as ejp,
            tc.tile_pool(name="psN", bufs=1, space="PSUM") as psN,
            tc.tile_pool(name="psC", bufs=4, space="PSUM") as psC,
            tc.tile_pool(name="psX", bufs=2, space="PSUM") as psX,
            tc.tile_pool(name="psD", bufs=2, space="PSUM") as psD,
        ):
            # ---------- prepass: normalize table, write DRAM augtab ----------
            embT_t = persist.tile([128, VPAD], bf16, name="embT_t")
            nc.sync.dma_start(out=embT_t[:], in_=embT_in[:, :])

            ones_t = persist.tile([128, 1], bf16, name="ones_t")
            nc.gpsimd.memset(ones_t[:], 1.0)

            # chunked table load (packed 264B rows, no junk)
            table_t = persist.tile([128, NSTRIPE, ROWP], bf16, name="table_t")
            WC = 10  # stripes per load/normalize/write chunk
            chunks = [(a0, min(a0 + WC, NSTRIPE)) for a0 in range(0, NSTRIPE, WC)]
            for a0, a1 in chunks:
                nc.sync.dma_start(
                    out=table_t[:, a0:a1, :].rearrange("p a r -> p (a r)"),
                    in_=table_in[:, ROWP * a0:ROWP * a1],
                )

            sq_t = persist.tile([128, VPAD], bf16, name="sq_t")
            idx_ts = []
            nrm2 = psN.tile([128, NSTRIPE], f32, name="nrm2", tag="nrm2")
            SQC = 20  # stripes per SQ chunk
            for a0 in range(0, NSTRIPE, SQC):
                a1 = min(a0 + SQC, NSTRIPE)
                nc.vector.tensor_tensor(
                    out=sq_t[:, 128 * a0:128 * a1], in0=embT_t[:, 128 * a0:128 * a1],
                    in1=embT_t[:, 128 * a0:128 * a1], op=ALU.mult,
                )
                for a in range(a0, a1):
                    nc.tensor.matmul(
                        nrm2[:, a:a + 1],
                        sq_t[:, 128 * a:128 * (a + 1)],
                        ones_t[:],
                        start=True, stop=True,
                    )
            sqn = persist.tile([128, NSTRIPE], f32, name="sqn")
            nc.scalar.sqrt(sqn[:], nrm2[:])
            invn = persist.tile([128, NSTRIPE], f32, name="invn")
            nc.vector.reciprocal(invn[:], sqn[:])

            aug_view = augtab[:, :].rearrange("(a p) r -> p a r", p=128)
            for a0, a1 in chunks:
                for a in range(a0, a1):
                    nc.vector.tensor_scalar_mul(
                        table_t[:, a, 0:H], table_t[:, a, 0:H], invn[:, a:a + 1]
                    )
                nc.sync.dma_start(
                    out=aug_view[:, a0:a1, 0:ROWP], in_=table_t[:, a0:a1, :]
                )

            # ---------- idx loads / constants ----------
            for b in range(NBATCH):
                idx_t = persist.tile([128, NIDX // 16], i16, name=f"idx_t{b}")
                nc.sync.dma_start(out=idx_t[:], in_=idx_in[b, :, :])
                idx_ts.append(idx_t)

            bias_t = persist.tile([128, 4], f32, name="bias_t")
            nc.sync.dma_start(out=bias_t[:], in_=biaspp[:, :])
            causal_t = persist.tile([128, 128], bf16, name="causal_t")
            nc.sync.dma_start(out=causal_t[:], in_=causal[:, :])
            identb_t = persist.tile([128, 128], bf16, name="identb_t")
            nc.sync.dma_start(out=identb_t[:], in_=identb[:, :])
            identf_t = persist.tile([128, 128], f32, name="identf_t")
            nc.sync.dma_start(out=identf_t[:], in_=identf[:, :])
            corr2_t = persist.tile([128, EXC * 4], bf16, name="corr2_t")
            nc.sync.dma_start(out=corr2_t[:], in_=corr2_in[:, :])

            F_all = persist.tile([128, 8 * EXC], f32, name="F_all")
            B_all = persist.tile([128, 4 * EXC], f32, name="B_all")

            # ---------- main loop: 2 batches x 8 examples ----------
            for b in range(NBATCH):
                G = main.tile([128, 2, NIDX], bf16, name="G", tag="G")
                nc.gpsimd.dma_gather(
                    G[:], augtab[:, :], idx_ts[b][:],
                    NIDX, NIDX, ROW,
                    transpose=True, single_packet=False,
                )

                # aux transposes for the whole batch:
                # psA cols [ex*24 + 4j + c] = hist aux (beta,r0,r1,pad) chunk j
                # psA cols [ex*24 + 16 + 2j] = next beta chunk j (2-col slots
                # keep PSUM matmul outputs 4-byte aligned)
                psA = psX.tile([128, EXB * 24], bf16, name="psA", tag="psA")
                for e8 in range(EXB):
                    o = 1024 * e8
                    for j in range(4):
                        nc.tensor.transpose(
                            psA[:, 24 * e8 + 4 * j:24 * e8 + 4 * j + 4],
                            G[0:4, 1, o + 128 * j:o + 128 * (j + 1)],
                            identb_t[0:4, 0:4],
                        )
                        nc.tensor.transpose(
                            psA[:, 24 * e8 + 16 + 2 * j:24 * e8 + 18 + 2 * j],
                            G[0:2, 1, o + 512 + 128 * j:o + 512 + 128 * (j + 1)],
                            identb_t[0:2, 0:2],
                        )
                sA = main.tile([128, EXB, 24], bf16, name="sA", tag="sA")
                nc.vector.tensor_copy(
                    sA[:].rearrange("p e c -> p (e c)"), psA[:]
                )
                r0v = sA[:, :, 1:16:4]
                r1v = sA[:, :, 2:16:4]
                padv = sA[:, :, 3:16:4]
                dm = main.tile([128, EXB * 4], bf16, name="dm", tag="dm")
                nc.vector.tensor_tensor(out=dm[:], in0=r1v, in1=r0v, op=ALU.subtract)
                mm = main.tile([128, EXB * 4], bf16, name="mm", tag="mm")
                nc.vector.tensor_tensor(
                    out=mm[:], in0=dm[:],
                    in1=corr2_t[:, 4 * EXB * b:4 * EXB * (b + 1)], op=ALU.mult,
                )
                mst = main.tile([128, EXB * 4], bf16, name="mst", tag="mst")
                nc.vector.tensor_tensor(out=mst[:], in0=mm[:], in1=r0v, op=ALU.add)
                # T4[p, e, j, 0] = mastery*pad ; T4[p, e, j, 1] = pad
                T_aux = main.tile([128, EXB, 4, 2], bf16, name="T_aux", tag="T_aux")
                nc.vector.tensor_tensor(
                    out=T_aux[:, :, :, 0], in0=mst[:].rearrange("p (e j) -> p e j", j=4),
                    in1=padv, op=ALU.mult,
                )
                nc.vector.tensor_copy(T_aux[:, :, :, 1], padv)
                nc.vector.tensor_copy(
                    B_all[:, 4 * EXB * b:4 * EXB * (b + 1)],
                    sA[:, :, 16:23:2],
                )

                for e8 in range(EXB):
                    e = EXB * b + e8
                    o = 1024 * e8
                    e_tiles = []
                    for j in range(4):
                        n_j = 512 - 128 * j
                        sc = psC.tile([128, 512], f32, name="sc", tag="sc", bufs=2)
                        nc.tensor.matmul(
                            sc[:, 0:n_j],
                            G[:, 0, o + 128 * j:o + 128 * (j + 1)],
                            G[:, 0, o + 512 + 128 * j:o + 1024],
                            start=True, stop=True,
                        )
                        e_j = ejp.tile([128, 512], bf16, name="e_j", tag=f"e_j{j}")
                        nc.scalar.activation(
                            e_j[:, 0:n_j], sc[:, 0:n_j], AF.Exp,
                            bias=bias_t[:, j:j + 1], scale=1.0,
                        )
                        nc.vector.tensor_tensor(
                            out=e_j[:, 0:128], in0=e_j[:, 0:128], in1=causal_t[:],
                            op=ALU.mult,
                        )
                        e_tiles.append(e_j)

                    nd = psD.tile([128, 8], f32, name="nd", tag="nd")
                    for c in range(4):
                        for j in range(c + 1):
                            nc.tensor.matmul(
                                nd[:, 2 * c:2 * c + 2],
                                e_tiles[j][:, 128 * (c - j):128 * (c - j + 1)],
                                T_aux[:, e8, j, :],
                                start=(j == 0), stop=(j == c),
                            )
                    nc.vector.tensor_copy(F_all[:, 8 * e:8 * e + 8], nd[:])

            # ---------- finals ----------
            F3 = F_all[:].rearrange("p (x t) -> p x t", t=2)
            rd = persist.tile([128, 64], f32, name="rd")
            nc.vector.reciprocal(rd[:], F3[:, :, 1])
            at = persist.tile([128, 64], f32, name="at")
            nc.vector.tensor_tensor(out=at[:], in0=F3[:, :, 0], in1=rd[:], op=ALU.mult)
            zt = persist.tile([128, 64], f32, name="zt")
            nc.vector.tensor_tensor(out=zt[:], in0=at[:], in1=B_all[:], op=ALU.subtract)
            ot = persist.tile([128, 64], f32, name="ot")
            nc.scalar.activation(ot[:], zt[:], AF.Sigmoid)
            pso = psN.tile([128, 128], f32, name="pso", tag="pso", bufs=1)
            nc.tensor.transpose(pso[0:64, :], ot[:], identf_t[:])
            otr = persist.tile([64, 128], f32, name="otr")
            nc.vector.tensor_copy(otr[:], pso[0:64, :])
            nc.sync.dma_start(
                out=out[:, :].rearrange("e (x q) -> (e x) q", x=4), in_=otr[:]
            )

    nc.finalize()
    return nc


def _marshal(inputs):
    import ml_dtypes

    bf16 = ml_dtypes.bfloat16
    hist = np.asarray(inputs["history_items"]).astype(np.int64)
    nxt = np.asarray(inputs["next_items"]).astype(np.int64)
    corrects = np.asarray(inputs["history_corrects"]).astype(np.int64)
    E = np.asarray(inputs["item_embedding"], dtype=np.float32)
    beta = np.asarray(inputs["item_beta_weights"], dtype=np.float32)
    resp = np.asarray(inputs["item_response_vals"], dtype=np.float32)
    k = float(np.asarray(inputs["td_kernel"]).reshape(-1)[0])

    emb_pad = np.ones((VPAD, H), dtype=np.float32)
    emb_pad[:V] = E
    emb16 = emb_pad.astype(bf16)

    # combined packed table, swizzled to [v%128 partitions, v//128 stripes, 132]
    table = np.zeros((VPAD, ROWP), dtype=bf16)
    table[:, 0:H] = emb16
    table[:V, H] = beta.astype(bf16)
    table[:V, H + 1] = resp[:, 0].astype(bf16)
    table[:V, H + 2] = resp[:, 1].astype(bf16)
    table[:V, H + 3] = 1.0
    table[0, H + 3] = 0.0  # item id 0 is padding
    table_sw = np.ascontiguousarray(
        table.reshape(NSTRIPE, 128, ROWP).transpose(1, 0, 2)
    ).reshape(128, NSTRIPE * ROWP)

    embT = np.ascontiguousarray(emb16.T)  # [H, VPAD]

    p = np.arange(128, dtype=np.float32)
    biaspp_np = np.stack(
        [-k * (128.0 * j + p) + k * (S / 2 - 0.5) for j in range(4)], axis=1
    ).astype(np.float32)

    causal_np = (p[:, None] <= p[None, :]).astype(bf16)  # keep s<=q within tile
    identb_np = np.eye(128, dtype=np.float32).astype(bf16)
    identf_np = np.eye(128, dtype=np.float32)

    in_maps = []
    for c in range(NCORES):
        idx_c = np.zeros((NBATCH, 128, NIDX // 16), dtype=np.int16)
        corr2_c = np.zeros((128, EXC * 4), dtype=np.float32)
        for b in range(NBATCH):
            ids = np.concatenate(
                [
                    np.concatenate([hist[c * EXC + EXB * b + e8], nxt[c * EXC + EXB * b + e8]])
                    for e8 in range(EXB)
                ]
            ).astype(np.int16)  # [8192]
            w = ids.reshape(NIDX // 16, 16).T  # [16, NIDX//16]
            idx_c[b] = np.tile(w, (8, 1))
        for e in range(EXC):
            cseq = (corrects[c * EXC + e].reshape(4, 128) == 2).astype(np.float32)
            corr2_c[:, 4 * e:4 * e + 4] = cseq.T
        in_maps.append(
            dict(
                table_in=table_sw,
                embT_in=embT,
                idx_in=idx_c,
                corr2_in=corr2_c.astype(bf16),
                biaspp=biaspp_np,
                causal=causal_np,
                identb=identb_np,
                identf=identf_np,
            )
        )
    return in_maps


def kernel(**inputs) -> np.ndarray:
    from concourse.bass_utils import run_bass_kernel_spmd

    if "nc" not in _CACHE:
        _CACHE["nc"] = _build_nc()
    nc = _CACHE["nc"]
    in_maps = _marshal(inputs)
    res = run_bass_kernel_spmd(nc, in_maps, list(range(NCORES))).results
    out = np.concatenate([res[c]["out"] for c in range(NCORES)], axis=0)
    return np.ascontiguousarray(out).astype(np.float32)
